# revision 29
# baseline (speedup 1.0000x reference)
"""Bahdanau attention kernel for 8 TRN2 NeuronCores.

Math: scores[q,k] = w2 . tanh(qW[q,:] + kW[k,:] + b1) (+ b2, dropped: softmax
is shift-invariant). The tanh over the [B,Q,K,A] tensor is replaced by a
separable product expansion fitted offline:

    tanh(x + y) ~= sum_j F_j(x) * psi_j(y),   F_j = w2 * sum_i C_ij phi_i(x)

The x-side combined functions F_j are folded on DVE (one op per nonzero C
entry + one w2-broadcast multiply) so the TensorEngine runs one contraction
group per y-function instead of one per (i,j) pair. b1 is folded into the kW
matmul as an extra rank-1 contraction chunk. Factor activations read qW/kW
straight from PSUM; tanh+sin live in one HW activation table
(silu_and_others) so there is a single table load (+1 for the final exp,
prefetched under the score matmuls). Softmax runs per k-half so
exp/mask/transpose/context overlap the score-matmul tail; masking is a -30
additive pre-exp term; no max subtraction (scores are bounded).

Sharding: data-parallel, core = (batch b, query-half qh); each core computes
a [128, 512] block of weights and context. Output: (context, weights).
"""

import numpy as np
import ml_dtypes

from contextlib import ExitStack
from concourse import bass, bacc, tile, mybir
from concourse.bass_utils import run_bass_kernel_spmd

BF16 = mybir.dt.bfloat16
F32 = mybir.dt.float32
AF = mybir.ActivationFunctionType
OP = mybir.AluOpType
NPBF = ml_dtypes.bfloat16

B, Q, K, H, A = 4, 256, 512, 512, 512
QSH = 128
N_CORES = 8
PH = float(np.pi / 4)
TMAX = 3.2          # |spline arg| budget for Sin
ALPHA = 1.5
MASK_NEG = -30.0

# ---- factor model (fitted offline; see fit.py / fit_run.py) ---------------
# atom spec: ('one',) | ('lin',) | ('tanh', a, mu) | ('sin', w, sgn)
#            | ('silu', a, mu) | ('relu', a, mu) | ('square', a, mu)
# J=4 tanh-only y-atoms, 5 used x-atoms (tanh/square), 10 nonzero C;
# everything (atoms + final exp) lives in one HW act table -> zero swaps.
# Fitted against measured HW activation profiles, validated end-to-end
# in numpy (incl bf16 fold effects): weights 4.4e-3 / context 5.0e-3.
XATOMS = [('one',), ('tanh', 2.0, 0.0), ('square', 1.0, 0.0),
          ('tanh', 2.4, 0.3), ('tanh', 2.4, -0.6), ('tanh', 2.4, 0.9),
          ('tanh', 1.6, -1.2), ('tanh', 1.3, 1.5), ('tanh', 1.0, 0.9),
          ('tanh', 1.3, -0.3), ('tanh', 1.0, -1.5)]
YATOMS = [('tanh', 1.0, 0.0), ('tanh', 1.0, 0.3), ('tanh', 1.0, -0.6),
          ('tanh', 1.0, 0.9)]
PAIRS = [
    (1, 0, 1.527000504744526),
    (1, 1, -2.040210898241307),
    (7, 0, 3.929565511809077),
    (7, 1, -3.0187012595237133),
    (7, 2, -1.626393563254583),
    (8, 0, -5.472665247610694),
    (8, 1, 3.556658593940851),
    (8, 2, 1.8627470384110558),
    (9, 0, 0.9376528535142259),
]
XMAX = 2.16
N_WARM = 12        # PE p-state warm-up matmuls bridging the DMA prologue


def _trig_clip(w):
    c = (TMAX - PH) / w
    return c if c < XMAX else None


def _consts_layout():
    cols = {('z',): 0}
    vals = [0.0]
    for spec in XATOMS + YATOMS:
        key = None
        bias = None
        if spec[0] in ('tanh', 'silu', 'relu', 'square'):
            key = (spec[0], spec[1], spec[2])
            bias = -spec[1] * spec[2]
        elif spec[0] == 'sin':
            key = ('s', spec[2])
            bias = PH * spec[2]
        if key is not None and key not in cols:
            cols[key] = len(vals)
            vals.append(bias)
    return cols, vals


CONSTS_COLS, CONSTS_VALS = _consts_layout()
NCONSTS = len(CONSTS_VALS)

AF_OF = {'tanh': AF.Tanh, 'sin': AF.Sin, 'silu': AF.Silu, 'relu': AF.Relu,
         'square': AF.Square}

# group pairs by y-atom: j -> [(i, c), ...]; 'one' terms handled at fold time
YGROUPS = {}
for (xi, yi, cf) in PAIRS:
    YGROUPS.setdefault(yi, []).append((xi, cf))
for yi in YGROUPS:
    YGROUPS[yi].sort(key=lambda t: t[0])
USED_X = sorted({p[0] for p in PAIRS})
Y_TANH = [j for j, s in enumerate(YATOMS)
          if s[0] in ('tanh', 'silu', 'relu', 'square') and j in YGROUPS]
Y_SIN = [j for j, s in enumerate(YATOMS) if s[0] == 'sin' and j in YGROUPS]
Y_LIN = [j for j, s in enumerate(YATOMS) if s[0] == 'lin' and j in YGROUPS]
if Y_TANH:
    Y_ORDER = [Y_TANH[0]] + Y_LIN + Y_TANH[1:] + Y_SIN
else:
    Y_ORDER = Y_LIN + Y_SIN


def _build_kernel():
    nc = bacc.Bacc("TRN2", target_bir_lowering=False, debug=False,
                   num_devices=N_CORES)

    d_qt = nc.declare_dram_parameter("qt", [128, 4 * QSH], BF16, isOutput=False)
    d_kt = nc.declare_dram_parameter("kt", [128, 4 * K], BF16, isOutput=False)
    d_v = nc.declare_dram_parameter("v", [128, 4 * H], BF16, isOutput=False)
    d_m = nc.declare_dram_parameter("m", [QSH, K], BF16, isOutput=False)
    d_w1a = nc.declare_dram_parameter("w1a", [128, 4 * A], BF16, isOutput=False)
    d_w1b = nc.declare_dram_parameter("w1b", [128, 4 * A], BF16, isOutput=False)
    d_b1r = nc.declare_dram_parameter("b1r", [1, A], BF16, isOutput=False)
    d_ones = nc.declare_dram_parameter("onesr", [1, K], BF16, isOutput=False)
    d_w2bc = nc.declare_dram_parameter("w2bc", [128, 512], BF16, isOutput=False)
    d_cb = nc.declare_dram_parameter("consts", [128, NCONSTS], F32,
                                     isOutput=False)
    d_id = nc.declare_dram_parameter("ident", [128, 128], BF16, isOutput=False)
    d_wout = nc.declare_dram_parameter("wout", [QSH, K], BF16, isOutput=True)
    d_cout = nc.declare_dram_parameter("cout", [QSH, H], BF16, isOutput=True)

    with tile.TileContext(nc) as tc, ExitStack() as ctx:
        sb = ctx.enter_context(tc.tile_pool(name="sb", bufs=1))
        ps_sc = ctx.enter_context(tc.tile_pool(name="pssc", bufs=1,
                                               space="PSUM"))
        pre_ctx = ExitStack()
        ps_pre = pre_ctx.enter_context(tc.tile_pool(name="pspre", bufs=1,
                                                    space="PSUM"))
        ps_dum = pre_ctx.enter_context(tc.tile_pool(name="psdum", bufs=1,
                                                    space="PSUM"))

        # ---- PE p-state warm-up: keep the systolic array busy while the
        # input DMAs land so the real matmuls run at full clock -----------
        dum_in = sb.tile([128, 128], BF16, tag="dum_in")
        nc.vector.memset(dum_in[:], 0.0)
        dum_mv = sb.tile([128, 512], BF16, tag="dum_mv")
        nc.vector.memset(dum_mv[:], 0.0)
        dum_ps = ps_dum.tile([128, 512], F32, tag="dum_ps")
        for _ in range(N_WARM):
            nc.tensor.matmul(dum_ps[:], dum_in[:], dum_mv[:],
                             start=True, stop=True)

        # ---- input DMAs: one rearranged DMA per tensor ------------------
        # sync queue: qWT path first, then kWT path; gpsimd: the rest
        w1bA = sb.tile([128, 4 * A], BF16, tag="w1bA")
        w1bB = sb.tile([128, 4 * A], BF16, tag="w1bB")
        qTs = sb.tile([128, 4 * QSH], BF16, tag="qTs")
        kTs = sb.tile([128, 4 * K], BF16, tag="kTs")
        cb = sb.tile([128, NCONSTS], F32, tag="cb")
        # strict order on one queue: the greedy PE scheduler runs whichever
        # matmul group's inputs land first, and qWT must win (the ACT queue
        # is in-order and x-atoms sit ahead of y-atoms). All tensors are
        # host-prepped contiguous [128, X] so descriptors are 2-4KB.
        nc.sync.dma_start(w1bA[:], d_w1a[:])
        nc.sync.dma_start(qTs[:], d_qt[:])
        nc.sync.dma_start(w1bB[:], d_w1b[:])
        nc.sync.dma_start(kTs[:], d_kt[:])
        vb = sb.tile([128, 4 * H], BF16, tag="vb")
        nc.sync.dma_start(vb[:], d_v[:])
        mf = sb.tile([128, K], BF16, tag="mf")
        nc.sync.dma_start(mf[:], d_m[:])
        ident = sb.tile([128, 128], BF16, tag="ident")
        nc.sync.dma_start(ident[:], d_id[:])
        nc.gpsimd.dma_start(cb[:], d_cb[:])
        b1r = sb.tile([1, A], BF16, tag="b1r")
        nc.gpsimd.dma_start(b1r[:], d_b1r[:])
        onesr = sb.tile([1, K], BF16, tag="onesr")
        nc.gpsimd.dma_start(onesr[:], d_ones[:])
        w2bc = sb.tile([128, 512], BF16, tag="w2bc")
        nc.gpsimd.dma_start(w2bc[:], d_w2bc[:])

        # ---- qWT [a, q] + b1 in PSUM (ab-outer: sequential acc groups;
        # b1 lives on the q side — only x+y matters and the rank-1 fold is
        # 4x cheaper here than on the 512-wide kWT groups) ----------------
        qwt_ps = ps_pre.tile([128, 512], F32, tag="qwt")
        for ab in range(4):
            for hc in range(4):
                nc.tensor.matmul(
                    qwt_ps[:, ab * 128:(ab + 1) * 128],
                    w1bA[:, hc * A + ab * 128: hc * A + (ab + 1) * 128],
                    qTs[:, hc * 128:(hc + 1) * 128],
                    start=(hc == 0), stop=False)
            nc.tensor.matmul(
                qwt_ps[:, ab * 128:(ab + 1) * 128],
                b1r[:, ab * 128:(ab + 1) * 128],
                onesr[:, 0:QSH],
                start=False, stop=True)

        # ---- kWT [a, k] in PSUM -----------------------------------------
        kwt_ps = ps_pre.tile([128, 2048], F32, tag="kwt")
        for ab in range(4):
            for hc in range(4):
                nc.tensor.matmul(
                    kwt_ps[:, ab * 512:(ab + 1) * 512],
                    w1bB[:, hc * A + ab * 128: hc * A + (ab + 1) * 128],
                    kTs[:, hc * 512:(hc + 1) * 512],
                    start=(hc == 0), stop=(hc == 3))

        # ---- atom emission helper ---------------------------------------
        def emit_atom(src, spec, out, clip_cache, width):
            kind = spec[0]
            if kind == 'sin':
                w = spec[1]
                c = _trig_clip(w)
                if c is not None:
                    ckey = ('clip', w)
                    if ckey not in clip_cache:
                        ct = sb.tile([128, width], F32,
                                     tag=f"clip{width}_{len(clip_cache)}")
                        nc.vector.tensor_scalar(ct[:], src[:], float(c),
                                                float(-c), OP.min, OP.max)
                        clip_cache[ckey] = ct
                    src = clip_cache[ckey]
                col = CONSTS_COLS[('s', spec[2])]
                nc.scalar.activation(out[:], src[:], AF.Sin,
                                     bias=cb[:, col:col + 1], scale=float(w))
            else:
                col = CONSTS_COLS[(kind, spec[1], spec[2])]
                nc.scalar.activation(out[:], src[:], AF_OF[kind],
                                     bias=cb[:, col:col + 1],
                                     scale=float(spec[1]))

        # ---- x-side atoms (tanh-family first, sins after clip) ----------
        xph = {}
        xclip_cache = {}
        x_order = ([i for i in USED_X if XATOMS[i][0] not in ('one', 'sin')] +
                   [i for i in USED_X if XATOMS[i][0] == 'sin'])
        for i in x_order:
            t = sb.tile([128, 512], BF16, tag=f"xf{i}")
            emit_atom(qwt_ps, XATOMS[i], t, xclip_cache, 512)
            xph[i] = t

        # ---- y lin atom (DVE cast from PSUM) ----------------------------
        yt = {}
        yclip_cache = {}
        for j in Y_LIN:
            t = sb.tile([128, 2048], BF16, tag=f"yf{j}")
            nc.vector.tensor_copy(t[:], kwt_ps[:])
            yt[j] = t

        # ---- y tanh-family atoms (ACT from PSUM) ------------------------
        # First atom is emitted per ab-chunk: each chunk only depends on its
        # kWT accumulation group, so it fills the ACT gap while the later
        # kWT groups are still running (bias is a plain constant now that
        # b1 lives on the q side). Remaining atoms go whole-tile (fewer
        # instruction bubbles once kwt is fully done).
        for n, j in enumerate(Y_TANH):
            t = sb.tile([128, 2048], BF16, tag=f"yf{j}")
            if n == 0:
                spec = YATOMS[j]
                col = CONSTS_COLS[(spec[0], spec[1], spec[2])]
                for ab in range(4):
                    sl = slice(ab * 512, (ab + 1) * 512)
                    nc.scalar.activation(t[:, sl], kwt_ps[:, sl],
                                         AF_OF[spec[0]],
                                         bias=cb[:, col:col + 1],
                                         scale=float(spec[1]))
            else:
                emit_atom(kwt_ps, YATOMS[j], t, yclip_cache, 2048)
            yt[j] = t

        # ---- F_j chains on DVE: raw-atom combo, then w2 broadcast fold --
        fts = {}
        for n, j in enumerate(Y_ORDER):
            terms = [(i, c) for (i, c) in YGROUPS[j] if XATOMS[i][0] != 'one']
            ones = [c for (i, c) in YGROUPS[j] if XATOMS[i][0] == 'one']
            c_one = float(sum(ones))
            ft = sb.tile([128, 512], BF16, tag=f"F{j}")
            if terms:
                acc = sb.tile([128, 512], BF16, tag=f"Fa{j}")
                i0, c0 = terms[0]
                nc.vector.tensor_scalar_mul(acc[:], xph[i0][:], float(c0))
                for (ii, cc) in terms[1:]:
                    nc.vector.scalar_tensor_tensor(acc[:], xph[ii][:],
                                                   float(cc), acc[:],
                                                   OP.mult, OP.add)
                if ones:
                    nc.vector.scalar_tensor_tensor(ft[:], acc[:], c_one,
                                                   w2bc[:], OP.add, OP.mult)
                else:
                    nc.vector.tensor_mul(ft[:], acc[:], w2bc[:])
            else:
                nc.vector.tensor_scalar_mul(ft[:], w2bc[:], c_one)
            fts[j] = ft

        # ---- y sin atoms (clip on DVE; ACT reads clipped SBUF) ----------
        for j in Y_SIN:
            t = sb.tile([128, 2048], BF16, tag=f"yf{j}")
            emit_atom(kwt_ps, YATOMS[j], t, yclip_cache, 2048)
            yt[j] = t

        # qwt/kwt PSUM banks are dead past this point; free them for tail
        pre_ctx.close()
        ps_tail = ctx.enter_context(tc.tile_pool(name="pstail", bufs=1,
                                                 space="PSUM"))
        ps_tp = ctx.enter_context(tc.tile_pool(name="pstp", bufs=2,
                                               space="PSUM"))

        # ---- score matmuls: per y-atom, k-half A then k-half B ----------
        sc_A_full = ps_sc.tile([128, 512], F32, tag="scA")
        sc_B_full = ps_sc.tile([128, 512], F32, tag="scB")
        sc_A = sc_A_full[:, 0:256]
        sc_B = sc_B_full[:, 0:256]
        nj = len(Y_ORDER)
        for n, j in enumerate(Y_ORDER):
            for half, sc in ((0, sc_A), (1, sc_B)):
                for ab in range(4):
                    nc.tensor.matmul(
                        sc[:],
                        fts[j][:, ab * 128:(ab + 1) * 128],
                        yt[j][:, ab * 512 + half * 256:
                              ab * 512 + half * 256 + 256],
                        start=(n == 0 and ab == 0),
                        stop=(n == nj - 1 and ab == 3))

        # exp table prefetch while the score matmuls drain (depends on the
        # last y atom so it schedules after all tanh/sin activations)
        dummy = sb.tile([128, 1], F32, tag="dummy")
        nc.scalar.activation(dummy[:], yt[Y_ORDER[-1]][:, 0:1], AF.Exp,
                             bias=0.0, scale=1.0)

        # ---- per-half masked softmax + context --------------------------
        ctx_ps = ps_tail.tile([128, 512], F32, tag="ctx")
        ssum = {}
        wexp = {}
        for half, sc in ((0, sc_A), (1, sc_B)):
            sm = sb.tile([128, 256], F32, tag=f"sm{half}")
            nc.vector.scalar_tensor_tensor(sm[:], mf[:, half * 256:
                                                     half * 256 + 256],
                                           MASK_NEG, sc[:], OP.mult, OP.add)
            we = sb.tile([128, 256], BF16, tag=f"we{half}")
            ss = sb.tile([128, 1], F32, tag=f"ss{half}")
            nc.scalar.activation(we[:], sm[:], AF.Exp, bias=0.0, scale=1.0,
                                 accum_out=ss[:])
            wexp[half] = we
            ssum[half] = ss
            wT = sb.tile([128, 256], BF16, tag=f"wT{half}")
            for i in range(2):
                pt = ps_tp.tile([128, 128], BF16, tag="tp")
                nc.tensor.transpose(pt[:], we[:, i * 128:(i + 1) * 128],
                                    ident[:])
                nc.vector.tensor_copy(wT[:, i * 128:(i + 1) * 128], pt[:])
            for i in range(2):
                kc = half * 2 + i
                nc.tensor.matmul(ctx_ps[:], wT[:, i * 128:(i + 1) * 128],
                                 vb[:, kc * 512:(kc + 1) * 512],
                                 start=(kc == 0), stop=(kc == 3))

        stot = sb.tile([128, 1], F32, tag="stot")
        nc.vector.tensor_add(stot[:], ssum[0][:], ssum[1][:])
        rinv = sb.tile([128, 1], F32, tag="rinv")
        nc.vector.reciprocal(rinv[:], stot[:])
        wout = sb.tile([128, 512], BF16, tag="wout")
        for half in (0, 1):
            nc.vector.tensor_scalar_mul(wout[:, half * 256:half * 256 + 256],
                                        wexp[half][:], rinv[:])
            nc.sync.dma_start(d_wout[:, half * 256:half * 256 + 256],
                              wout[:, half * 256:half * 256 + 256])
        cout = sb.tile([128, 512], BF16, tag="cout")
        for ch, q in ((0, nc.scalar), (1, nc.gpsimd)):
            sl = slice(ch * 256, ch * 256 + 256)
            nc.vector.tensor_scalar_mul(cout[:, sl], ctx_ps[:, sl], rinv[:])
            q.dma_start(d_cout[:, sl], cout[:, sl])

    nc.compile()
    return nc


_NC_CACHE = None


def _get_nc():
    global _NC_CACHE
    if _NC_CACHE is None:
        _NC_CACHE = _build_kernel()
    return _NC_CACHE


def _host_inputs(query, keys, values, mask, W1, b1, w2, b2):
    query = np.asarray(query, np.float32).astype(NPBF)
    keys = np.asarray(keys, np.float32).astype(NPBF)
    values = np.asarray(values, np.float32).astype(NPBF)
    maskb = np.asarray(mask).astype(NPBF)
    W1 = np.ascontiguousarray(np.asarray(W1, np.float32).astype(NPBF))
    b1 = np.asarray(b1, np.float32)
    w2 = np.asarray(w2, np.float32)
    b1r = np.ascontiguousarray(b1.astype(NPBF).reshape(1, A))
    onesr = np.ones((1, K), dtype=NPBF)
    w2cc = np.ascontiguousarray(w2.reshape(4, 128).T.astype(np.float32))
    w2bc = np.ascontiguousarray(
        np.repeat(w2cc.astype(NPBF)[:, :, None], 128, axis=2).reshape(128, 512))
    consts = np.zeros((128, NCONSTS), np.float32)
    for c, v in enumerate(CONSTS_VALS):
        consts[:, c] = v
    ident = np.eye(128, dtype=NPBF)

    def chunk128(mat, width):
        """[512, width] -> [128, 4*width]: chunk rows to partitions."""
        return np.ascontiguousarray(
            mat.reshape(4, 128, width).transpose(1, 0, 2).reshape(128,
                                                                  4 * width))

    w1a_p = chunk128(W1[:H], A)
    w1b_p = chunk128(W1[H:], A)

    in_maps = []
    for c in range(N_CORES):
        b, qh = c // 2, c % 2
        qT = np.ascontiguousarray(query[b, qh * QSH:(qh + 1) * QSH, :].T)
        in_maps.append({
            "qt": chunk128(qT, QSH),
            "kt": chunk128(np.ascontiguousarray(keys[b].T), K),
            "v": chunk128(values[b], H),
            "m": np.ascontiguousarray(maskb[b, qh * QSH:(qh + 1) * QSH, :]),
            "w1a": w1a_p,
            "w1b": w1b_p,
            "b1r": b1r,
            "onesr": onesr,
            "w2bc": w2bc,
            "consts": consts,
            "ident": ident,
        })
    return in_maps


def _run(inputs, trace=False, **kw):
    nc = _get_nc()
    in_maps = _host_inputs(**inputs)
    res = run_bass_kernel_spmd(nc, in_maps, list(range(N_CORES)),
                               trace=trace, **kw)
    context = np.zeros((B, Q, H), np.float32)
    weights = np.zeros((B, Q, K), np.float32)
    for c in range(N_CORES):
        b, qh = c // 2, c % 2
        weights[b, qh * QSH:(qh + 1) * QSH, :] = \
            res.results[c]["wout"].astype(np.float32)
        context[b, qh * QSH:(qh + 1) * QSH, :] = \
            res.results[c]["cout"].astype(np.float32)
    return (context, weights), res


def kernel(query, keys, values, mask, W1, b1, w2, b2):
    (context, weights), _ = _run(dict(query=query, keys=keys, values=values,
                                      mask=mask, W1=W1, b1=b1, w2=w2, b2=b2))
    return context, weights


# revision 31
# speedup vs baseline: 1.0148x; 1.0148x over previous
"""Bahdanau attention kernel for 8 TRN2 NeuronCores.

Math: scores[q,k] = w2 . tanh(qW[q,:] + kW[k,:] + b1) (+ b2, dropped: softmax
is shift-invariant). The tanh over the [B,Q,K,A] tensor is replaced by a
separable product expansion fitted offline:

    tanh(x + y) ~= sum_j F_j(x) * psi_j(y),   F_j = w2 * sum_i C_ij phi_i(x)

The x-side combined functions F_j are folded on DVE (one op per nonzero C
entry + one w2-broadcast multiply) so the TensorEngine runs one contraction
group per y-function instead of one per (i,j) pair. b1 is folded into the kW
matmul as an extra rank-1 contraction chunk. Factor activations read qW/kW
straight from PSUM; tanh+sin live in one HW activation table
(silu_and_others) so there is a single table load (+1 for the final exp,
prefetched under the score matmuls). Softmax runs per k-half so
exp/mask/transpose/context overlap the score-matmul tail; masking is a -30
additive pre-exp term; no max subtraction (scores are bounded).

Sharding: data-parallel, core = (batch b, query-half qh); each core computes
a [128, 512] block of weights and context. Output: (context, weights).
"""

import numpy as np
import ml_dtypes

from contextlib import ExitStack
from concourse import bass, bacc, tile, mybir
from concourse.bass_utils import run_bass_kernel_spmd

BF16 = mybir.dt.bfloat16
F32 = mybir.dt.float32
AF = mybir.ActivationFunctionType
OP = mybir.AluOpType
NPBF = ml_dtypes.bfloat16

B, Q, K, H, A = 4, 256, 512, 512, 512
QSH = 128
N_CORES = 8
PH = float(np.pi / 4)
TMAX = 3.2          # |spline arg| budget for Sin
ALPHA = 1.5
MASK_NEG = -30.0

# ---- factor model (fitted offline; see fit.py / fit_run.py) ---------------
# atom spec: ('one',) | ('lin',) | ('tanh', a, mu) | ('sin', w, sgn)
#            | ('silu', a, mu) | ('relu', a, mu) | ('square', a, mu)
# J=4 tanh-only y-atoms, 5 used x-atoms (tanh/square), 10 nonzero C;
# everything (atoms + final exp) lives in one HW act table -> zero swaps.
# Fitted against measured HW activation profiles, validated end-to-end
# in numpy (incl bf16 fold effects): weights 4.4e-3 / context 5.0e-3.
XATOMS = [('one',), ('tanh', 2.0, 0.0), ('square', 1.0, 0.0),
          ('tanh', 2.4, 0.3), ('tanh', 2.4, -0.6), ('tanh', 2.4, 0.9),
          ('tanh', 1.6, -1.2), ('tanh', 1.3, 1.5), ('tanh', 1.0, 0.9),
          ('tanh', 1.3, -0.3), ('tanh', 1.0, -1.5)]
YATOMS = [('tanh', 1.0, 0.0), ('tanh', 1.0, 0.3), ('tanh', 1.0, -0.6),
          ('tanh', 1.0, 0.9)]
PAIRS = [
    (1, 0, 1.527000504744526),
    (1, 1, -2.040210898241307),
    (7, 0, 3.929565511809077),
    (7, 1, -3.0187012595237133),
    (7, 2, -1.626393563254583),
    (8, 0, -5.472665247610694),
    (8, 1, 3.556658593940851),
    (8, 2, 1.8627470384110558),
    (9, 0, 0.9376528535142259),
]
XMAX = 2.16
N_WARM = 10        # PE p-state warm-up matmuls bridging the DMA prologue


def _trig_clip(w):
    c = (TMAX - PH) / w
    return c if c < XMAX else None


def _consts_layout():
    cols = {('z',): 0}
    vals = [0.0]
    for spec in XATOMS + YATOMS:
        key = None
        bias = None
        if spec[0] in ('tanh', 'silu', 'relu', 'square'):
            key = (spec[0], spec[1], spec[2])
            bias = -spec[1] * spec[2]
        elif spec[0] == 'sin':
            key = ('s', spec[2])
            bias = PH * spec[2]
        if key is not None and key not in cols:
            cols[key] = len(vals)
            vals.append(bias)
    return cols, vals


CONSTS_COLS, CONSTS_VALS = _consts_layout()
NCONSTS = len(CONSTS_VALS)

AF_OF = {'tanh': AF.Tanh, 'sin': AF.Sin, 'silu': AF.Silu, 'relu': AF.Relu,
         'square': AF.Square}

# group pairs by y-atom: j -> [(i, c), ...]; 'one' terms handled at fold time
YGROUPS = {}
for (xi, yi, cf) in PAIRS:
    YGROUPS.setdefault(yi, []).append((xi, cf))
for yi in YGROUPS:
    YGROUPS[yi].sort(key=lambda t: t[0])
USED_X = sorted({p[0] for p in PAIRS})
Y_TANH = [j for j, s in enumerate(YATOMS)
          if s[0] in ('tanh', 'silu', 'relu', 'square') and j in YGROUPS]
Y_SIN = [j for j, s in enumerate(YATOMS) if s[0] == 'sin' and j in YGROUPS]
Y_LIN = [j for j, s in enumerate(YATOMS) if s[0] == 'lin' and j in YGROUPS]
if Y_TANH:
    Y_ORDER = [Y_TANH[0]] + Y_LIN + Y_TANH[1:] + Y_SIN
else:
    Y_ORDER = Y_LIN + Y_SIN


def _build_kernel():
    nc = bacc.Bacc("TRN2", target_bir_lowering=False, debug=False,
                   num_devices=N_CORES)

    d_qt = nc.declare_dram_parameter("qt", [128, 4 * QSH], BF16, isOutput=False)
    d_kt = nc.declare_dram_parameter("kt", [128, 4 * K], BF16, isOutput=False)
    d_v = nc.declare_dram_parameter("v", [128, 4 * H], BF16, isOutput=False)
    d_m = nc.declare_dram_parameter("m", [QSH, K], BF16, isOutput=False)
    d_w1a = nc.declare_dram_parameter("w1a", [128, 4 * A], BF16, isOutput=False)
    d_w1b = nc.declare_dram_parameter("w1b", [128, 4 * A], BF16, isOutput=False)
    d_b1r = nc.declare_dram_parameter("b1r", [1, A], BF16, isOutput=False)
    d_ones = nc.declare_dram_parameter("onesr", [1, K], BF16, isOutput=False)
    d_w2bc = nc.declare_dram_parameter("w2bc", [128, 512], BF16, isOutput=False)
    d_cb = nc.declare_dram_parameter("consts", [128, NCONSTS], F32,
                                     isOutput=False)
    d_id = nc.declare_dram_parameter("ident", [128, 128], BF16, isOutput=False)
    d_wout = nc.declare_dram_parameter("wout", [QSH, K], BF16, isOutput=True)
    d_cout = nc.declare_dram_parameter("cout", [QSH, H], BF16, isOutput=True)

    with tile.TileContext(nc) as tc, ExitStack() as ctx:
        sb = ctx.enter_context(tc.tile_pool(name="sb", bufs=1))
        ps_sc = ctx.enter_context(tc.tile_pool(name="pssc", bufs=1,
                                               space="PSUM"))
        pre_ctx = ExitStack()
        ps_pre = pre_ctx.enter_context(tc.tile_pool(name="pspre", bufs=1,
                                                    space="PSUM"))
        ps_dum = pre_ctx.enter_context(tc.tile_pool(name="psdum", bufs=1,
                                                    space="PSUM"))

        # ---- PE p-state warm-up: keep the systolic array busy while the
        # input DMAs land so the real matmuls run at full clock -----------
        dum_in = sb.tile([128, 128], BF16, tag="dum_in")
        nc.vector.memset(dum_in[:], 0.0)
        dum_mv = sb.tile([128, 512], BF16, tag="dum_mv")
        nc.vector.memset(dum_mv[:], 0.0)
        dum_ps = ps_dum.tile([128, 512], F32, tag="dum_ps")
        for _ in range(N_WARM):
            nc.tensor.matmul(dum_ps[:], dum_in[:], dum_mv[:],
                             start=True, stop=True)

        # ---- input DMAs: one rearranged DMA per tensor ------------------
        # sync queue: qWT path first, then kWT path; gpsimd: the rest
        w1bA = sb.tile([128, 4 * A], BF16, tag="w1bA")
        w1bB = sb.tile([128, 4 * A], BF16, tag="w1bB")
        qTs = sb.tile([128, 4 * QSH], BF16, tag="qTs")
        kTs = sb.tile([128, 4 * K], BF16, tag="kTs")
        cb = sb.tile([128, NCONSTS], F32, tag="cb")
        # strict order on one queue: the greedy PE scheduler runs whichever
        # matmul group's inputs land first, and qWT must win (the ACT queue
        # is in-order and x-atoms sit ahead of y-atoms). All tensors are
        # host-prepped contiguous [128, X] so descriptors are 2-4KB.
        nc.sync.dma_start(w1bA[:], d_w1a[:])
        nc.sync.dma_start(qTs[:], d_qt[:])
        nc.sync.dma_start(w1bB[:], d_w1b[:])
        nc.sync.dma_start(kTs[:], d_kt[:])
        vb = sb.tile([128, 4 * H], BF16, tag="vb")
        nc.sync.dma_start(vb[:], d_v[:])
        mf = sb.tile([128, K], BF16, tag="mf")
        nc.sync.dma_start(mf[:], d_m[:])
        ident = sb.tile([128, 128], BF16, tag="ident")
        nc.sync.dma_start(ident[:], d_id[:])
        nc.gpsimd.dma_start(cb[:], d_cb[:])
        b1r = sb.tile([1, A], BF16, tag="b1r")
        nc.gpsimd.dma_start(b1r[:], d_b1r[:])
        onesr = sb.tile([1, K], BF16, tag="onesr")
        nc.gpsimd.dma_start(onesr[:], d_ones[:])
        w2bc = sb.tile([128, 512], BF16, tag="w2bc")
        nc.gpsimd.dma_start(w2bc[:], d_w2bc[:])

        # ---- qWT [a, q] + b1 in PSUM (ab-outer: sequential acc groups;
        # b1 lives on the q side — only x+y matters and the rank-1 fold is
        # 4x cheaper here than on the 512-wide kWT groups) ----------------
        qwt_ps = ps_pre.tile([128, 512], F32, tag="qwt")
        for ab in range(4):
            for hc in range(4):
                nc.tensor.matmul(
                    qwt_ps[:, ab * 128:(ab + 1) * 128],
                    w1bA[:, hc * A + ab * 128: hc * A + (ab + 1) * 128],
                    qTs[:, hc * 128:(hc + 1) * 128],
                    start=(hc == 0), stop=False)
            nc.tensor.matmul(
                qwt_ps[:, ab * 128:(ab + 1) * 128],
                b1r[:, ab * 128:(ab + 1) * 128],
                onesr[:, 0:QSH],
                start=False, stop=True)

        # ---- kWT [a, k] in PSUM -----------------------------------------
        kwt_ps = ps_pre.tile([128, 2048], F32, tag="kwt")
        for ab in range(4):
            for hc in range(4):
                nc.tensor.matmul(
                    kwt_ps[:, ab * 512:(ab + 1) * 512],
                    w1bB[:, hc * A + ab * 128: hc * A + (ab + 1) * 128],
                    kTs[:, hc * 512:(hc + 1) * 512],
                    start=(hc == 0), stop=(hc == 3))

        # ---- atom emission helper ---------------------------------------
        def emit_atom(src, spec, out, clip_cache, width):
            kind = spec[0]
            if kind == 'sin':
                w = spec[1]
                c = _trig_clip(w)
                if c is not None:
                    ckey = ('clip', w)
                    if ckey not in clip_cache:
                        ct = sb.tile([128, width], F32,
                                     tag=f"clip{width}_{len(clip_cache)}")
                        nc.vector.tensor_scalar(ct[:], src[:], float(c),
                                                float(-c), OP.min, OP.max)
                        clip_cache[ckey] = ct
                    src = clip_cache[ckey]
                col = CONSTS_COLS[('s', spec[2])]
                nc.scalar.activation(out[:], src[:], AF.Sin,
                                     bias=cb[:, col:col + 1], scale=float(w))
            else:
                col = CONSTS_COLS[(kind, spec[1], spec[2])]
                nc.scalar.activation(out[:], src[:], AF_OF[kind],
                                     bias=cb[:, col:col + 1],
                                     scale=float(spec[1]))

        # ---- x-side atoms (tanh-family first, sins after clip) ----------
        xph = {}
        xclip_cache = {}
        x_order = ([i for i in USED_X if XATOMS[i][0] not in ('one', 'sin')] +
                   [i for i in USED_X if XATOMS[i][0] == 'sin'])
        for i in x_order:
            t = sb.tile([128, 512], BF16, tag=f"xf{i}")
            emit_atom(qwt_ps, XATOMS[i], t, xclip_cache, 512)
            xph[i] = t

        # ---- y lin atom (DVE cast from PSUM) ----------------------------
        yt = {}
        yclip_cache = {}
        for j in Y_LIN:
            t = sb.tile([128, 2048], BF16, tag=f"yf{j}")
            nc.vector.tensor_copy(t[:], kwt_ps[:])
            yt[j] = t

        # ---- y tanh-family atoms (ACT from PSUM) ------------------------
        for j in Y_TANH:
            t = sb.tile([128, 2048], BF16, tag=f"yf{j}")
            emit_atom(kwt_ps, YATOMS[j], t, yclip_cache, 2048)
            yt[j] = t

        # ---- F_j chains on DVE: raw-atom combo, then w2 broadcast fold --
        fts = {}
        for n, j in enumerate(Y_ORDER):
            terms = [(i, c) for (i, c) in YGROUPS[j] if XATOMS[i][0] != 'one']
            ones = [c for (i, c) in YGROUPS[j] if XATOMS[i][0] == 'one']
            c_one = float(sum(ones))
            ft = sb.tile([128, 512], BF16, tag=f"F{j}")
            if terms:
                acc = sb.tile([128, 512], BF16, tag=f"Fa{j}")
                i0, c0 = terms[0]
                nc.vector.tensor_scalar_mul(acc[:], xph[i0][:], float(c0))
                for (ii, cc) in terms[1:]:
                    nc.vector.scalar_tensor_tensor(acc[:], xph[ii][:],
                                                   float(cc), acc[:],
                                                   OP.mult, OP.add)
                if ones:
                    nc.vector.scalar_tensor_tensor(ft[:], acc[:], c_one,
                                                   w2bc[:], OP.add, OP.mult)
                else:
                    nc.vector.tensor_mul(ft[:], acc[:], w2bc[:])
            else:
                nc.vector.tensor_scalar_mul(ft[:], w2bc[:], c_one)
            fts[j] = ft

        # ---- y sin atoms (clip on DVE; ACT reads clipped SBUF) ----------
        for j in Y_SIN:
            t = sb.tile([128, 2048], BF16, tag=f"yf{j}")
            emit_atom(kwt_ps, YATOMS[j], t, yclip_cache, 2048)
            yt[j] = t

        # qwt/kwt PSUM banks are dead past this point; free them for tail
        pre_ctx.close()
        ps_tail = ctx.enter_context(tc.tile_pool(name="pstail", bufs=1,
                                                 space="PSUM"))
        ps_tp = ctx.enter_context(tc.tile_pool(name="pstp", bufs=2,
                                               space="PSUM"))

        # ---- score matmuls: per y-atom, k-half A then k-half B ----------
        sc_A_full = ps_sc.tile([128, 512], F32, tag="scA")
        sc_B_full = ps_sc.tile([128, 512], F32, tag="scB")
        sc_A = sc_A_full[:, 0:256]
        sc_B = sc_B_full[:, 0:256]
        nj = len(Y_ORDER)
        for n, j in enumerate(Y_ORDER):
            for half, sc in ((0, sc_A), (1, sc_B)):
                for ab in range(4):
                    nc.tensor.matmul(
                        sc[:],
                        fts[j][:, ab * 128:(ab + 1) * 128],
                        yt[j][:, ab * 512 + half * 256:
                              ab * 512 + half * 256 + 256],
                        start=(n == 0 and ab == 0),
                        stop=(n == nj - 1 and ab == 3))

        # exp table prefetch while the score matmuls drain (depends on the
        # last y atom so it schedules after all tanh/sin activations)
        dummy = sb.tile([128, 1], F32, tag="dummy")
        nc.scalar.activation(dummy[:], yt[Y_ORDER[-1]][:, 0:1], AF.Exp,
                             bias=0.0, scale=1.0)

        # ---- per-half masked softmax + context --------------------------
        ctx_ps = ps_tail.tile([128, 512], F32, tag="ctx")
        ssum = {}
        wexp = {}
        for half, sc in ((0, sc_A), (1, sc_B)):
            sm = sb.tile([128, 256], F32, tag=f"sm{half}")
            nc.vector.scalar_tensor_tensor(sm[:], mf[:, half * 256:
                                                     half * 256 + 256],
                                           MASK_NEG, sc[:], OP.mult, OP.add)
            we = sb.tile([128, 256], BF16, tag=f"we{half}")
            ss = sb.tile([128, 1], F32, tag=f"ss{half}")
            nc.scalar.activation(we[:], sm[:], AF.Exp, bias=0.0, scale=1.0,
                                 accum_out=ss[:])
            wexp[half] = we
            ssum[half] = ss
            wT = sb.tile([128, 256], BF16, tag=f"wT{half}")
            for i in range(2):
                pt = ps_tp.tile([128, 128], BF16, tag="tp")
                nc.tensor.transpose(pt[:], we[:, i * 128:(i + 1) * 128],
                                    ident[:])
                nc.vector.tensor_copy(wT[:, i * 128:(i + 1) * 128], pt[:])
            for i in range(2):
                kc = half * 2 + i
                nc.tensor.matmul(ctx_ps[:], wT[:, i * 128:(i + 1) * 128],
                                 vb[:, kc * 512:(kc + 1) * 512],
                                 start=(kc == 0), stop=(kc == 3))

        stot = sb.tile([128, 1], F32, tag="stot")
        nc.vector.tensor_add(stot[:], ssum[0][:], ssum[1][:])
        rinv = sb.tile([128, 1], F32, tag="rinv")
        nc.vector.reciprocal(rinv[:], stot[:])
        wout = sb.tile([128, 512], BF16, tag="wout")
        for half, q in ((0, nc.sync), (1, nc.gpsimd)):
            nc.vector.tensor_scalar_mul(wout[:, half * 256:half * 256 + 256],
                                        wexp[half][:], rinv[:])
            q.dma_start(d_wout[:, half * 256:half * 256 + 256],
                        wout[:, half * 256:half * 256 + 256])
        cout = sb.tile([128, 512], BF16, tag="cout")
        for ch, q in ((0, nc.scalar), (1, nc.sync)):
            sl = slice(ch * 256, ch * 256 + 256)
            nc.vector.tensor_scalar_mul(cout[:, sl], ctx_ps[:, sl], rinv[:])
            q.dma_start(d_cout[:, sl], cout[:, sl])

    nc.compile()
    return nc


_NC_CACHE = None


def _get_nc():
    global _NC_CACHE
    if _NC_CACHE is None:
        _NC_CACHE = _build_kernel()
    return _NC_CACHE


def _host_inputs(query, keys, values, mask, W1, b1, w2, b2):
    query = np.asarray(query, np.float32).astype(NPBF)
    keys = np.asarray(keys, np.float32).astype(NPBF)
    values = np.asarray(values, np.float32).astype(NPBF)
    maskb = np.asarray(mask).astype(NPBF)
    W1 = np.ascontiguousarray(np.asarray(W1, np.float32).astype(NPBF))
    b1 = np.asarray(b1, np.float32)
    w2 = np.asarray(w2, np.float32)
    b1r = np.ascontiguousarray(b1.astype(NPBF).reshape(1, A))
    onesr = np.ones((1, K), dtype=NPBF)
    w2cc = np.ascontiguousarray(w2.reshape(4, 128).T.astype(np.float32))
    w2bc = np.ascontiguousarray(
        np.repeat(w2cc.astype(NPBF)[:, :, None], 128, axis=2).reshape(128, 512))
    consts = np.zeros((128, NCONSTS), np.float32)
    for c, v in enumerate(CONSTS_VALS):
        consts[:, c] = v
    ident = np.eye(128, dtype=NPBF)

    def chunk128(mat, width):
        """[512, width] -> [128, 4*width]: chunk rows to partitions."""
        return np.ascontiguousarray(
            mat.reshape(4, 128, width).transpose(1, 0, 2).reshape(128,
                                                                  4 * width))

    w1a_p = chunk128(W1[:H], A)
    w1b_p = chunk128(W1[H:], A)

    in_maps = []
    for c in range(N_CORES):
        b, qh = c // 2, c % 2
        qT = np.ascontiguousarray(query[b, qh * QSH:(qh + 1) * QSH, :].T)
        in_maps.append({
            "qt": chunk128(qT, QSH),
            "kt": chunk128(np.ascontiguousarray(keys[b].T), K),
            "v": chunk128(values[b], H),
            "m": np.ascontiguousarray(maskb[b, qh * QSH:(qh + 1) * QSH, :]),
            "w1a": w1a_p,
            "w1b": w1b_p,
            "b1r": b1r,
            "onesr": onesr,
            "w2bc": w2bc,
            "consts": consts,
            "ident": ident,
        })
    return in_maps


def _run(inputs, trace=False, **kw):
    nc = _get_nc()
    in_maps = _host_inputs(**inputs)
    res = run_bass_kernel_spmd(nc, in_maps, list(range(N_CORES)),
                               trace=trace, **kw)
    context = np.zeros((B, Q, H), np.float32)
    weights = np.zeros((B, Q, K), np.float32)
    for c in range(N_CORES):
        b, qh = c // 2, c % 2
        weights[b, qh * QSH:(qh + 1) * QSH, :] = \
            res.results[c]["wout"].astype(np.float32)
        context[b, qh * QSH:(qh + 1) * QSH, :] = \
            res.results[c]["cout"].astype(np.float32)
    return (context, weights), res


def kernel(query, keys, values, mask, W1, b1, w2, b2):
    (context, weights), _ = _run(dict(query=query, keys=keys, values=values,
                                      mask=mask, W1=W1, b1=b1, w2=w2, b2=b2))
    return context, weights
